# revision 20
# baseline (speedup 1.0000x reference)
"""GraphSAGE v12: per-tile gathers round-robined on 4 SWDGE queues,
two-chunk overlapped allgather, tile-major phase 2.

SWDGE descriptor emission (~7.9ns/row per queue) is the wall for row
gathers; per-tile gather instructions (1408 rows) rotating across all 4
queues keep every Q7 pair emitting concurrently (~2.6ns/row aggregate),
with the Pool engine merely blocking on the oldest queue. Each
collective_compute costs ~25us of mostly size-independent latency and
they serialize on the CC core, and all late tiles' shard writes land
within a few us of each other -- so the h1 allgather is exactly two
chunks: the first half's collective runs under phase 1 (issued with
enough slack that its input wait is satisfied), the whole second half
goes in one final collective. Phase-2 refs are tile-major so each of
the 4 gathers feeds one output tile's add-tree/transpose/matmul
pipeline directly.
"""

import sys

for _p in ("/opt/trn_rl_repo", "/root/.axon_site/_ro/trn_rl_repo"):
    if _p not in sys.path:
        sys.path.insert(0, _p)

import numpy as np

import concourse.bass as bass
import concourse.mybir as mybir
import concourse.tile as tile
from concourse import bacc
from concourse.bass import BassGpSimd
from concourse.bass_utils import run_bass_kernel_spmd

N, D, OUT, K = 100000, 256, 128, 10
N1, B = 40960, 4096
NCORES = 8
BC = B // NCORES                 # 512 batch rows per core
NREF = BC * (K + 1)              # 5632 phase-2 refs
TR = NREF // 128                 # 44 phase-2 gather columns
T2 = BC // 128                   # 4 output tiles
K1 = K + 1
CT = 32768                       # compacted local table rows (int16 max)
NQ = 4                           # SWDGE queues (max)

COLL_ENGINE = "gpsimd"           # "scalar" | "gpsimd"

_CACHE = {}


def _chunk_tiles(t1):
    """Chunk boundaries in tiles. Each collective costs ~25us of mostly
    size-independent latency and they serialize on the CC core, and the
    late tiles' shard writes all land within a few us of each other --
    so exactly two chunks: the first half allgathers under phase 1, the
    whole second half goes in one final collective."""
    if t1 < 6:
        return [t1]
    c0 = (t1 * 5 + 4) // 9
    sizes = [c0, t1 - c0]
    assert sum(sizes) == t1 and all(s > 0 for s in sizes)
    return sizes


def _build(SH):
    T1 = SH // 128
    U = SH * NCORES
    assert U <= 32767, U
    CH_T = _chunk_tiles(T1)                       # chunk sizes in tiles
    CH_START = [sum(CH_T[:i]) for i in range(len(CH_T))]
    CH_END = [s + c for s, c in zip(CH_START, CH_T)]
    # issue each chunk's collective SLACK tiles after the chunk ends, so
    # its input wait is already satisfied and never stalls the gather
    # stream; late chunks' collectives park at the end of the stream
    coll_after_tile = {T1 - 1: list(range(len(CH_T)))}
    NIX1 = SH * K1
    f32 = mybir.dt.float32
    f16 = mybir.dt.float16
    i16 = mybir.dt.int16
    idfun = mybir.ActivationFunctionType.Identity
    relu = mybir.ActivationFunctionType.Relu
    nc = bacc.Bacc("TRN2", target_bir_lowering=False, debug=False,
                   num_devices=NCORES, num_swdge_queues=NQ)
    ctable = nc.dram_tensor("ctable", [CT, D], f16, kind="ExternalInput").ap()
    ids1 = nc.dram_tensor("ids1", [128, NIX1 // 16], i16,
                          kind="ExternalInput").ap()
    ids2 = nc.dram_tensor("ids2", [128, NREF // 16], i16,
                          kind="ExternalInput").ap()
    w1p = nc.dram_tensor("w1p", [2 * D, OUT], f16, kind="ExternalInput").ap()
    w2p = nc.dram_tensor("w2p", [2 * OUT, OUT], f16, kind="ExternalInput").ap()
    ident = nc.dram_tensor("ident", [128, 128], f16, kind="ExternalInput").ap()
    out = nc.dram_tensor("out", [BC, OUT], f32, kind="ExternalOutput").ap()
    shard = nc.dram_tensor("shard", [SH, OUT], f16)
    h1all = nc.dram_tensor("h1all", [U, OUT], f16, addr_space="Shared")

    def coll(c):
        st = CH_START[c] * 128
        L = CH_T[c] * 128
        eng = nc.scalar if COLL_ENGINE == "scalar" else nc.gpsimd
        BassGpSimd.collective_compute(
            eng,
            "AllGather", mybir.AluOpType.bypass,
            replica_groups=[list(range(NCORES))],
            ins=[shard[st:st + L, :]],
            outs=[h1all[st * NCORES:(st + L) * NCORES, :]],
        )

    with tile.TileContext(nc) as tc:
        with tc.tile_pool(name="const", bufs=1) as constp, \
             tc.tile_pool(name="gat", bufs=16) as gatp, \
             tc.tile_pool(name="tree", bufs=3) as treep, \
             tc.tile_pool(name="agg", bufs=4) as aggp, \
             tc.tile_pool(name="xt", bufs=8) as xtp, \
             tc.tile_pool(name="g2", bufs=1) as g2p, \
             tc.tile_pool(name="ps", bufs=4, space="PSUM") as psp, \
             tc.tile_pool(name="psh", bufs=3, space="PSUM") as pshp, \
             tc.tile_pool(name="ps2", bufs=1, space="PSUM") as ps2p, \
             tc.tile_pool(name="o", bufs=4) as outp:

            ids1_all = constp.tile([128, NIX1 // 16], i16, tag="ids1")
            nc.sync.dma_start(out=ids1_all[:], in_=ids1[:])
            idn = constp.tile([128, 128], f16)
            nc.sync.dma_start(out=idn[:], in_=ident[:])
            w1t = constp.tile([128, 4 * OUT], f16, tag="w1")
            for c in range(4):
                nc.sync.dma_start(out=w1t[:, c * OUT:(c + 1) * OUT],
                                  in_=w1p[c * 128:(c + 1) * 128, :])
            w2t = constp.tile([128, 2 * OUT], f16, tag="w2")
            for c in range(2):
                nc.sync.dma_start(out=w2t[:, c * OUT:(c + 1) * OUT],
                                  in_=w2p[c * 128:(c + 1) * 128, :])
            ids2_all = constp.tile([128, NREF // 16], i16, tag="ids2")
            nc.sync.dma_start(out=ids2_all[:], in_=ids2[:, :])

            # ---- phase 1: per-tile gathers, round-robin queues ----
            for t in range(T1):
                g = gatp.tile([128, K1 * D], f16, tag="g")
                nc.gpsimd.dma_gather(
                    out_ap=g[:].rearrange("p (q e) -> p q e", e=D),
                    in_ap=ctable[:],
                    idxs_ap=ids1_all[:, t * K1 * 8:(t + 1) * K1 * 8],
                    num_idxs=K1 * 128, num_idxs_reg=K1 * 128,
                    elem_size=D, transpose=False, single_packet=False,
                    queue_num=t % NQ)
                s = treep.tile([128, 5 * D], f16, tag="s")
                nc.vector.tensor_add(s[:], g[:, D:6 * D], g[:, 6 * D:11 * D])
                t2 = treep.tile([128, 2 * D], f16, tag="t2")
                nc.vector.tensor_add(t2[:], s[:, 0:2 * D], s[:, 2 * D:4 * D])
                a = aggp.tile([128, D], f16)
                nc.vector.tensor_add(a[:], t2[:, 0:D], t2[:, D:2 * D])
                nc.vector.tensor_add(a[:], a[:], s[:, 4 * D:5 * D])
                srcs = (g[:, 0:128], g[:, 128:256],
                        a[:, 0:128], a[:, 128:256])
                psum_h = pshp.tile([128, 128], f32, space="PSUM")
                for c, src in enumerate(srcs):
                    pt = psp.tile([128, 128], f16, space="PSUM", tag="tp")
                    nc.tensor.transpose(out=pt[:], in_=src, identity=idn[:])
                    xt = xtp.tile([128, 128], f16, tag=f"xt{c}")
                    nc.vector.tensor_copy(out=xt[:], in_=pt[:])
                    nc.tensor.matmul(out=psum_h[:],
                                     lhsT=xt[:],
                                     rhs=w1t[:, c * OUT:(c + 1) * OUT],
                                     start=(c == 0), stop=(c == 3))
                ho = outp.tile([128, OUT], f16, tag="ho")
                nc.scalar.activation(ho[:], psum_h[:], relu)

                nc.sync.dma_start(out=shard[t * 128:(t + 1) * 128, :],
                                  in_=ho[:])
                for c in coll_after_tile.get(t, ()):
                    coll(c)

            # ---- phase 2: 4 parallel gathers, then second layer ----
            g2 = g2p.tile([128, TR * OUT], f16)
            for t in range(T2):
                q0 = t * K1
                nc.gpsimd.dma_gather(
                    out_ap=g2[:, q0 * OUT:(q0 + K1) * OUT]
                        .rearrange("p (q e) -> p q e", e=OUT),
                    in_ap=h1all[:],
                    idxs_ap=ids2_all[:, q0 * 8:(q0 + K1) * 8],
                    num_idxs=K1 * 128, num_idxs_reg=K1 * 128,
                    elem_size=OUT, transpose=False, single_packet=False,
                    queue_num=t % NQ)

            # refs layout: col (t*11+u)*128+d, u=0 self, u=k+1 neighbor k
            for t in range(T2):
                base = t * K1 * OUT
                s = treep.tile([128, 5 * 128], f16, tag="s2")
                nc.vector.tensor_add(s[:], g2[:, base + OUT:base + 6 * OUT],
                                     g2[:, base + 6 * OUT:base + 11 * OUT])
                t3 = treep.tile([128, 2 * 128], f16, tag="t3")
                nc.vector.tensor_add(t3[:], s[:, 0:256], s[:, 256:512])
                a2 = aggp.tile([128, 128], f16, tag="a2")
                nc.vector.tensor_add(a2[:], t3[:, 0:128], t3[:, 128:256])
                nc.vector.tensor_add(a2[:], a2[:], s[:, 512:640])
                ps2 = ps2p.tile([128, 128], f32, space="PSUM", tag="ps2")
                st = psp.tile([128, 128], f16, space="PSUM", tag="tp")
                nc.tensor.transpose(out=st[:],
                                    in_=g2[:, base:base + OUT],
                                    identity=idn[:])
                s2t = xtp.tile([128, 128], f16, tag="s2t")
                nc.vector.tensor_copy(out=s2t[:], in_=st[:])
                at = psp.tile([128, 128], f16, space="PSUM", tag="tp")
                nc.tensor.transpose(out=at[:], in_=a2[:], identity=idn[:])
                a2t = xtp.tile([128, 128], f16, tag="a2t")
                nc.vector.tensor_copy(out=a2t[:], in_=at[:])
                nc.tensor.matmul(out=ps2[:], lhsT=s2t[:], rhs=w2t[:, 0:OUT],
                                 start=True, stop=False)
                nc.tensor.matmul(out=ps2[:], lhsT=a2t[:],
                                 rhs=w2t[:, OUT:2 * OUT],
                                 start=False, stop=True)
                o = outp.tile([128, OUT], f32, tag="o2")
                nc.scalar.activation(o[:], ps2[:], relu)
                nc.sync.dma_start(out=out[t * 128:(t + 1) * 128, :], in_=o[:])

    nc.compile()
    return nc


def _wrap16(l):
    """dma_gather idx layout: idx[16j+p, s] = l[s*16+p], replicated to all
    8 Q7 partition groups."""
    l = np.asarray(l).astype(np.int16)
    assert len(l) % 16 == 0
    return np.ascontiguousarray(
        np.tile(l.reshape(-1, 16).T, (8, 1)).astype(np.int16))


def _prep_inputs(raw_features, W1, W2, nodes1, neighs1, map2, neighs2):
    raw16 = np.asarray(raw_features, dtype=np.float32).astype(np.float16)
    W1 = np.asarray(W1, dtype=np.float32)
    W2 = np.asarray(W2, dtype=np.float32)
    nodes1 = np.asarray(nodes1).astype(np.int64)
    neighs1 = np.asarray(neighs1).astype(np.int64)
    map2 = np.asarray(map2).astype(np.int64)
    neighs2 = np.asarray(neighs2).astype(np.int64)

    w1p = np.concatenate([W1[:, :D], W1[:, D:] * (1.0 / K)], axis=1).T
    w2p = np.concatenate([W2[:, :OUT], W2[:, OUT:] * (1.0 / K)], axis=1).T
    w1p = np.ascontiguousarray(w1p).astype(np.float16)
    w2p = np.ascontiguousarray(w2p).astype(np.float16)
    ident = np.eye(128, dtype=np.float16)

    refs = np.concatenate([map2, neighs2.reshape(-1)])      # [45056]
    uniq, inv = np.unique(refs, return_inverse=True)
    ua = len(uniq)
    SH = -(-ua // (NCORES * 128)) * 128
    T1 = SH // 128
    U = SH * NCORES
    CH_T = _chunk_tiles(T1)
    CHUNKS = [c * 128 for c in CH_T]
    CH_START = tuple(sum(CHUNKS[:i]) for i in range(len(CHUNKS)))
    uniq_pad = np.concatenate([uniq, np.zeros(U - ua, dtype=uniq.dtype)])
    cidx = np.arange(U) // SH
    r = np.arange(U) % SH
    starts = np.asarray(CH_START)
    sizes = np.asarray(CHUNKS)
    j = np.searchsorted(starts, r, side="right") - 1
    pos_of_u = starts[j] * NCORES + cidx * sizes[j] + (r - starts[j])

    in_maps = []
    for c in range(NCORES):
        blk = uniq_pad[c * SH:(c + 1) * SH]
        R = np.concatenate([nodes1[blk][:, None], neighs1[blk]], axis=1)
        luniq, linv = np.unique(R, return_inverse=True)
        assert len(luniq) <= CT, len(luniq)
        linv = linv.reshape(SH, K1)
        ctab = np.zeros((CT, D), dtype=np.float16)
        ctab[:len(luniq)] = raw16[luniq]
        # phase-1 idx order, tile t: i = u*128 + p, node = t*128 + p
        rows = linv.reshape(T1, 128, K1).transpose(0, 2, 1)   # [T1, 11, 128]
        ids1m = _wrap16(rows.reshape(-1))
        # phase-2 refs: col q = t*11+u; i = q*128 + p (tile-major so
        # gather j == output tile j and its compute pipelines)
        parts2 = []
        for t in range(T2):
            b0 = c * BC + t * 128
            rows_b = np.arange(b0, b0 + 128)
            parts2.append(pos_of_u[inv[rows_b]])
            nb = inv[B + (rows_b[:, None] * K + np.arange(K)[None, :])]
            parts2.append(pos_of_u[nb.T.reshape(-1)])
        ids2m = _wrap16(np.concatenate(parts2))
        in_maps.append({"ctable": ctab, "ids1": ids1m, "ids2": ids2m,
                        "w1p": w1p, "w2p": w2p, "ident": ident})
    return SH, in_maps


def run(inputs: dict, trace: bool = False):
    SH, in_maps = _prep_inputs(**inputs)
    if SH not in _CACHE:
        _CACHE[SH] = _build(SH)
    nc = _CACHE[SH]
    try:
        res = run_bass_kernel_spmd(nc, in_maps,
                                   core_ids=list(range(NCORES)), trace=trace)
    except Exception:
        res = run_bass_kernel_spmd(nc, in_maps,
                                   core_ids=list(range(NCORES)), trace=trace)
    outp = np.concatenate([res.results[c]["out"] for c in range(NCORES)],
                          axis=0)
    return outp.astype(np.float32), res.exec_time_ns


def kernel(**inputs) -> np.ndarray:
    out, _ = run(inputs, trace=False)
    return out
